# revision 5
# baseline (speedup 1.0000x reference)
"""Trainium2 Bass kernel for an AttentionBlock:
  qkv projections -> per-head softmax attention -> 2-layer relu MLP on ctx
  -> residual(+x) -> LayerNorm.

Full (unsharded) inputs in / full output out.  Internally sharded over 8
NeuronCores: core c handles (batch b=c//2, sequence half h=c%2) -- it computes
the output for its 1024 query tokens, recomputing K/V for the full 2048-token
batch locally so no collectives are needed.  The host reorders each core's x
so its query tokens are always rows 0:1024 (attention is permutation
invariant over keys, so K/V ordering does not matter).

Matmuls run in bf16 with fp32 PSUM accumulation; softmax (exp) and LayerNorm
run in fp32.  Scores are computed transposed [k, q] so the softmax
denominator comes from an extra ones-column appended to V (one fused matmul),
and the 1/denominator is applied to ctx via a GPSIMD partition-broadcast.
"""

import os
import sys

sys.path.insert(0, "/opt/trn_rl_repo")

import numpy as np

import concourse.bass as bass
import concourse.tile as tile
from concourse import bacc, mybir
from concourse.bass_utils import run_bass_kernel_spmd
from concourse.masks import make_identity

P = 128
B, S, D = 4, 2048, 768
H, DH = 12, 64
HID = 2 * D
EPS = 1e-5
NCORES = 8
SH = S // 2  # query tokens per core

DC = D // P  # 6 d-chunks of 128
TC = S // P  # 16 key-token chunks
HB = HID // P  # 12 hidden blocks
NQB = SH // 512  # 2 query blocks of 512
HPAIRS = H // 2  # 6 head pairs

F32 = mybir.dt.float32
BF16 = mybir.dt.bfloat16
ALU = mybir.AluOpType
AF = mybir.ActivationFunctionType


def _bcast_ap(ap, parts):
    """DRAM AP [n] -> [parts, n] with 0-stride partition broadcast."""
    return bass.AP(tensor=ap.tensor, offset=ap.offset, ap=[[0, parts], *ap.ap])


def build_kernel():
    nc = bacc.Bacc(
        "TRN2",
        target_bir_lowering=False,
        debug=False,
        enable_asserts=False,
        num_devices=NCORES,
    )

    x_d = nc.dram_tensor("x", [S, D], F32, kind="ExternalInput").ap()
    wq_d = nc.dram_tensor("Wq", [D, D], F32, kind="ExternalInput").ap()
    wk_d = nc.dram_tensor("Wk", [D, D], F32, kind="ExternalInput").ap()
    wv_d = nc.dram_tensor("Wv", [D, D], F32, kind="ExternalInput").ap()
    bq_d = nc.dram_tensor("bq", [D], F32, kind="ExternalInput").ap()
    bk_d = nc.dram_tensor("bk", [D], F32, kind="ExternalInput").ap()
    bv_d = nc.dram_tensor("bv", [D], F32, kind="ExternalInput").ap()
    w1_d = nc.dram_tensor("W1", [D, HID], F32, kind="ExternalInput").ap()
    b1_d = nc.dram_tensor("b1", [HID], F32, kind="ExternalInput").ap()
    w2_d = nc.dram_tensor("W2", [HID, D], F32, kind="ExternalInput").ap()
    b2_d = nc.dram_tensor("b2", [D], F32, kind="ExternalInput").ap()
    gamma_d = nc.dram_tensor("gamma", [D], F32, kind="ExternalInput").ap()
    beta_d = nc.dram_tensor("beta", [D], F32, kind="ExternalInput").ap()
    y_d = nc.dram_tensor("y", [SH, D], F32, kind="ExternalOutput").ap()

    x_r = x_d.rearrange("(o p) d -> p o d", p=P)  # [128, 16, 768]
    y_r = y_d.rearrange("(o p) d -> p o d", p=P)  # [128, 8, 768]
    wq_r = wq_d.rearrange("(o p) n -> p o n", p=P)  # [128, 6, 768]
    wk_r = wk_d.rearrange("(o p) n -> p o n", p=P)
    wv_r = wv_d.rearrange("(o p) n -> p o n", p=P)
    w1_r = w1_d.rearrange("(o p) n -> p o n", p=P)  # [128, 6, 1536]
    w2_r = w2_d.rearrange("(o p) n -> p o n", p=P)  # [128, 12, 768]

    from contextlib import ExitStack

    with tile.TileContext(nc) as tc, ExitStack() as ctx:
        persist = ctx.enter_context(tc.tile_pool(name="persist", bufs=1))
        ps = ctx.enter_context(tc.tile_pool(name="ps", bufs=4, space="PSUM"))
        ps_s = ctx.enter_context(tc.tile_pool(name="ps_s", bufs=2, space="PSUM"))

        # ---- constants -------------------------------------------------
        ident_f = persist.tile([P, P], F32)
        make_identity(nc, ident_f)
        ones_col = persist.tile([1, P], BF16)
        nc.vector.memset(ones_col[:], 1.0)
        eps_t = persist.tile([P, 1], F32)
        nc.vector.memset(eps_t[:], EPS)

        bq_c = persist.tile([P, DC], F32)
        nc.sync.dma_start(bq_c[:], bq_d.rearrange("(o p) -> p o", p=P))
        bk_c = persist.tile([P, DC], F32)
        nc.sync.dma_start(bk_c[:], bk_d.rearrange("(o p) -> p o", p=P))
        b1_c = persist.tile([P, HB], F32)
        nc.sync.dma_start(b1_c[:], b1_d.rearrange("(o p) -> p o", p=P))
        b2_c = persist.tile([P, DC], F32)
        nc.sync.dma_start(b2_c[:], b2_d.rearrange("(o p) -> p o", p=P))
        bv_row = persist.tile([1, D], BF16)
        gamma_bc = persist.tile([P, D], F32)
        nc.sync.dma_start(gamma_bc[:], _bcast_ap(gamma_d, P))
        beta_bc = persist.tile([P, D], F32)
        nc.sync.dma_start(beta_bc[:], _bcast_ap(beta_d, P))

        # ---- persistent activations / weights --------------------------
        qt = persist.tile([P, DC, SH], BF16)  # Q^T  [d, q]
        kt = persist.tile([P, DC, S], BF16)  # K^T  [d, k]
        # V augmented per head-pair: [Ve(64) | ones | zeros(62) | ones | Vo(64)]
        vag = persist.tile([P, TC, HPAIRS, 192], BF16)
        ctxt = persist.tile([P, DC, SH], BF16)  # ctx^T [d, q], normalized
        w1_b = persist.tile([P, DC, HID], BF16)
        w2_b = persist.tile([P, HB, D], BF16)

        nc.vector.memset(vag[:, :, :, 65:128], 0.0)
        nc.vector.memset(vag[:, :, :, 64:65], 1.0)

        with ExitStack() as phase1:
            qkvpool = phase1.enter_context(tc.tile_pool(name="qkvpool", bufs=1))
            wslot = phase1.enter_context(tc.tile_pool(name="wslot", bufs=1))
            stg = phase1.enter_context(tc.tile_pool(name="stg", bufs=1))

            bv_st = stg.tile([1, D], F32, tag="bvst")
            nc.sync.dma_start(bv_st[:], bv_d.rearrange("(a d) -> a d", a=1))
            nc.vector.tensor_copy(bv_row[:], bv_st[:])

            # ---- load x, transpose to xT (bf16) ------------------------
            xT = qkvpool.tile([P, DC, S], BF16)
            for quarter in range(4):
                xn = stg.tile([P, 4, D], F32, tag="xn")
                nc.sync.dma_start(xn[:], x_r[:, quarter * 4 : quarter * 4 + 4, :])
                for o in range(4):
                    to = quarter * 4 + o
                    for kc in range(DC):
                        pt = ps.tile([P, 512], F32, tag="ps", name="pt")[:, 0:P]
                        nc.tensor.transpose(
                            pt, xn[:, o, kc * P : (kc + 1) * P], ident_f
                        )
                        nc.vector.tensor_copy(
                            xT[:, kc, to * P : (to + 1) * P], pt
                        )

            def load_w_bf(dst, src_r, chunks):
                # stage fp32 -> cast bf16 into dst (same [p, o, n] layout)
                for sl_o, sl_n in chunks:
                    st = stg.tile(
                        [P, sl_o.stop - sl_o.start, sl_n.stop - sl_n.start],
                        F32,
                        tag="wstage",
                    )
                    nc.sync.dma_start(st[:], src_r[:, sl_o, sl_n])
                    nc.vector.tensor_copy(dst[:, sl_o, sl_n], st[:])

            # ---- QKV projections --------------------------------------
            wq_b = wslot.tile([P, DC, D], BF16, tag="w")
            load_w_bf(wq_b, wq_r, [(slice(0, DC), slice(0, 384)), (slice(0, DC), slice(384, D))])
            for mc in range(DC):
                for qb in range(NQB):
                    pq = ps.tile([P, 512], F32, tag="ps")
                    for kc in range(DC):
                        nc.tensor.matmul(
                            pq[:],
                            wq_b[:, kc, mc * P : (mc + 1) * P],
                            xT[:, kc, qb * 512 : (qb + 1) * 512],
                            start=(kc == 0),
                            stop=(kc == DC - 1),
                        )
                    nc.vector.tensor_scalar_add(
                        qt[:, mc, qb * 512 : (qb + 1) * 512],
                        pq[:],
                        bq_c[:, mc : mc + 1],
                    )

            wk_b = wslot.tile([P, DC, D], BF16, tag="w")
            load_w_bf(wk_b, wk_r, [(slice(0, DC), slice(0, 384)), (slice(0, DC), slice(384, D))])
            for mc in range(DC):
                for kb in range(S // 512):
                    pk = ps.tile([P, 512], F32, tag="ps")
                    for kc in range(DC):
                        nc.tensor.matmul(
                            pk[:],
                            wk_b[:, kc, mc * P : (mc + 1) * P],
                            xT[:, kc, kb * 512 : (kb + 1) * 512],
                            start=(kc == 0),
                            stop=(kc == DC - 1),
                        )
                    nc.vector.tensor_scalar_add(
                        kt[:, mc, kb * 512 : (kb + 1) * 512],
                        pk[:],
                        bk_c[:, mc : mc + 1],
                    )

            wv_b = wslot.tile([P, DC, D], BF16, tag="w")
            load_w_bf(wv_b, wv_r, [(slice(0, DC), slice(0, 384)), (slice(0, DC), slice(384, D))])
            for to in range(TC):
                for nb in range(2):
                    pv = ps.tile([P, 512], F32, tag="ps", name="pv")[:, 0:384]
                    for kc in range(DC):
                        nc.tensor.matmul(
                            pv,
                            xT[:, kc, to * P : (to + 1) * P],
                            wv_b[:, kc, nb * 384 : (nb + 1) * 384],
                            start=(kc == 0),
                            stop=False,
                        )
                    nc.tensor.matmul(
                        pv,
                        ones_col[0:1, 0:P],
                        bv_row[0:1, nb * 384 : (nb + 1) * 384],
                        start=False,
                        stop=True,
                    )
                    pv3 = pv.rearrange("p (a b) -> p a b", a=3)
                    nc.vector.tensor_copy(
                        vag[:, to, 3 * nb : 3 * nb + 3, 0:64], pv3[:, :, 0:64]
                    )
                    nc.vector.tensor_copy(
                        vag[:, to, 3 * nb : 3 * nb + 3, 128:192],
                        pv3[:, :, 64:128],
                    )

            # ---- MLP weights (emitted here; scheduler overlaps) --------
            load_w_bf(
                w1_b,
                w1_r,
                [(slice(0, DC), slice(j * 384, (j + 1) * 384)) for j in range(4)],
            )
            load_w_bf(
                w2_b,
                w2_r,
                [(slice(3 * j, 3 * j + 3), slice(0, D)) for j in range(4)],
            )

        # ---- attention -------------------------------------------------
        attn_ctx = ExitStack()
        exppool = attn_ctx.enter_context(tc.tile_pool(name="exppool", bufs=1))
        att = attn_ctx.enter_context(tc.tile_pool(name="att", bufs=2))
        for hc in range(HPAIRS):
            for qb in range(NQB):
                qsl = slice(qb * 512, (qb + 1) * 512)
                expt = exppool.tile([P, TC, 2, 512], BF16, tag="exp")
                for kb in range(TC):
                    psc = ps_s.tile([P, 2, 512], F32, tag="sc")
                    for e in range(2):
                        nc.tensor.matmul(
                            psc[:, e, :],
                            kt[e * 64 : (e + 1) * 64, hc, kb * P : (kb + 1) * P],
                            qt[e * 64 : (e + 1) * 64, hc, qsl],
                            start=True,
                            stop=True,
                        )
                    nc.scalar.activation(
                        expt[:, kb, :, :], psc[:], AF.Exp, scale=0.125
                    )
                for e in range(2):
                    pc = ps.tile([P, 512], F32, tag="ps")
                    if e == 0:
                        out_sl = pc[0:65, :]
                        lh_sl = slice(0, 65)
                        den_row = 64
                        ctx_rows = slice(0, 64)
                    else:
                        out_sl = pc[0:128, :]
                        lh_sl = slice(64, 192)
                        den_row = 0
                        ctx_rows = slice(64, 128)
                    for kb in range(TC):
                        nc.tensor.matmul(
                            out_sl,
                            vag[:, kb, hc, lh_sl],
                            expt[:, kb, e, :],
                            start=(kb == 0),
                            stop=(kb == TC - 1),
                        )
                    rec = att.tile([P, 512], F32, tag="rec")
                    nc.vector.reciprocal(
                        rec[den_row : den_row + 1, :], pc[den_row : den_row + 1, :]
                    )
                    rbc = att.tile([P, 512], F32, tag="rbc")
                    nc.gpsimd.partition_broadcast(
                        rbc[ctx_rows, :], rec[den_row : den_row + 1, :]
                    )
                    nc.vector.tensor_tensor(
                        ctxt[ctx_rows, hc, qsl], pc[ctx_rows, :], rbc[ctx_rows, :], ALU.mult
                    )

        attn_ctx.close()

        # ---- MLP + residual + LayerNorm -------------------------------
        mlp_ctx = ExitStack()
        mlp = mlp_ctx.enter_context(tc.tile_pool(name="mlp", bufs=1))
        mlp2 = mlp_ctx.enter_context(tc.tile_pool(name="mlp2", bufs=2))
        ln = mlp_ctx.enter_context(tc.tile_pool(name="ln", bufs=2))
        for qb in range(NQB):
            qsl = slice(qb * 512, (qb + 1) * 512)
            h1 = mlp.tile([P, HB, 512], BF16, tag="h1")
            for hb in range(HB):
                ph = ps.tile([P, 512], F32, tag="ps")
                for kc in range(DC):
                    nc.tensor.matmul(
                        ph[:],
                        w1_b[:, kc, hb * P : (hb + 1) * P],
                        ctxt[:, kc, qsl],
                        start=(kc == 0),
                        stop=(kc == DC - 1),
                    )
                nc.vector.tensor_scalar(
                    h1[:, hb, :], ph[:], b1_c[:, hb : hb + 1], 0.0, ALU.add, ALU.max
                )
            o2 = mlp2.tile([P, DC, 512], F32, tag="o2")
            for mc in range(DC):
                po = ps.tile([P, 512], F32, tag="ps")
                for hb in range(HB):
                    nc.tensor.matmul(
                        po[:],
                        w2_b[:, hb, mc * P : (mc + 1) * P],
                        h1[:, hb, :],
                        start=(hb == 0),
                        stop=(hb == HB - 1),
                    )
                nc.vector.tensor_scalar_add(
                    o2[:, mc, :], po[:], b2_c[:, mc : mc + 1]
                )
            for j in range(4):
                oc = qb * 4 + j
                xq = ln.tile([P, D], F32, tag="xq")
                nc.sync.dma_start(xq[:], x_r[:, oc, :])
                hsb = ln.tile([P, D], F32, tag="h")
                for kc in range(DC):
                    pt = ps.tile([P, 512], F32, tag="ps", name="pt")[:, 0:P]
                    nc.tensor.transpose(
                        pt, o2[:, kc, j * P : (j + 1) * P], ident_f
                    )
                    nc.vector.tensor_add(
                        hsb[:, kc * P : (kc + 1) * P], pt, xq[:, kc * P : (kc + 1) * P]
                    )
                stats = ln.tile([P, 2, 6], F32, tag="st")
                for g in range(2):
                    nc.vector.bn_stats(stats[:, g, :], hsb[:, g * 384 : (g + 1) * 384])
                mv = ln.tile([P, 2], F32, tag="mv")
                nc.vector.bn_aggr(mv[:], stats[:])
                sd = ln.tile([P, 2], F32, tag="sd")
                nc.scalar.activation(
                    sd[:, 0:1], mv[:, 1:2], AF.Sqrt, bias=eps_t[:, 0:1]
                )
                nc.vector.reciprocal(sd[:, 1:2], sd[:, 0:1])
                ysb = ln.tile([P, D], F32, tag="y")
                nc.vector.tensor_scalar(
                    ysb[:], hsb[:], mv[:, 0:1], sd[:, 1:2], ALU.subtract, ALU.mult
                )
                nc.vector.tensor_tensor(ysb[:], ysb[:], gamma_bc[:], ALU.mult)
                nc.vector.tensor_add(ysb[:], ysb[:], beta_bc[:])
                nc.sync.dma_start(y_r[:, oc, :], ysb[:])
        mlp_ctx.close()

    nc.compile()
    return nc


_NC_CACHE = None


def _get_nc():
    global _NC_CACHE
    if _NC_CACHE is None:
        _NC_CACHE = build_kernel()
    return _NC_CACHE


def make_in_maps(inputs):
    x = np.asarray(inputs["x"], dtype=np.float32)
    shared = {
        k: np.ascontiguousarray(np.asarray(inputs[k], dtype=np.float32))
        for k in ("Wq", "Wk", "Wv", "bq", "bk", "bv", "W1", "b1", "W2", "b2", "gamma", "beta")
    }
    in_maps = []
    for c in range(NCORES):
        b, half = c // 2, c % 2
        xq = x[b, half * SH : (half + 1) * SH]
        xo = x[b, (1 - half) * SH : (2 - half) * SH]
        xc = np.ascontiguousarray(np.concatenate([xq, xo], axis=0))
        in_maps.append({"x": xc, **shared})
    return in_maps


def kernel(**inputs) -> np.ndarray:
    nc = _get_nc()
    in_maps = make_in_maps(inputs)
    res = run_bass_kernel_spmd(nc, in_maps, core_ids=list(range(NCORES)))
    out = np.empty((B, S, D), dtype=np.float32)
    for c in range(NCORES):
        b, half = c // 2, c % 2
        out[b, half * SH : (half + 1) * SH] = res.results[c]["y"]
    return out
